# revision 8
# baseline (speedup 1.0000x reference)
"""Trainium2 Bass kernel for ContextQueryAttention (BiDAF-style trilinear attention).

Computes, per batch n:
    sim[c,q] = <ctx[c], wc> + <xq[q], wc> + <ctx[c] * wcq, xq[q]>
    c2q  = softmax_q(sim) @ xq                      # [C, F]
    q2c  = softmax_c(max_q sim) @ ctx               # [F]
    out  = concat([ctx, c2q, ctx*c2q, ctx*q2c], -1) # [C, 4F]

Sharding: data-parallel over batch N=64 across 8 NeuronCores (8 batches/core).

fp16 datapath (1 cyc/row on the PE vs 4 for fp32), fp32 PSUM accumulation and
fp32 softmax math -> total l2 ~2e-3 vs the fp32 reference (bf16 is 1.6e-2,
too close to the 2e-2 gate: sim logits span +-220, softmax gaps are O(0.1)).

I/O diet (~35MB/core instead of 86MB):
  - inputs host-cast to fp16
  - output term 1 is x_context verbatim -> host-assembled from the input;
    the device computes and stores only terms 2-4 as fp16 [C, 3F]

v3: instruction-count diet. The v2 trace showed ACT/DVE latency-bound at
~480ns/instr (248 ACT + 430 DVE instrs) and 49us of gpsimd tensor muls:
  - one [128, CT, 3F] asm tile and ONE output store per batch
  - term3 / term4 are single whole-batch DVE ops (term4 uses a stride-0
    broadcast AP of the q2c row)
  - sim psum tiles hold 2 c-tiles per bank -> nmax / z-extract ops halved
  - reciprocal batched over all CT row-sums (accum_out lands in columns)
  - xqw built with one broadcast DVE mul + one strided wc-column copy
  - ctxT psum uses full banks: 8 transposes then one [128, 1024] copy
  - gpsimd only does the cross-partition max reduce
"""

import os

os.environ.setdefault("JAX_PLATFORMS", "axon")

import numpy as np

import concourse.bass as bass
import concourse.mybir as mybir
import concourse.tile as tile
from concourse import bacc, bass_isa, bass_utils
from concourse.masks import make_identity

f32 = mybir.dt.float32
f16 = mybir.dt.float16
AX = mybir.AxisListType.X
EXP = mybir.ActivationFunctionType.Exp
COPY = mybir.ActivationFunctionType.Copy
MULT = mybir.AluOpType.mult
ADD = mybir.AluOpType.add

N_CORES = 8
N = 64         # total batches
B = 8          # batches per core
C = 1024       # context length
Q = 128        # query length
F = 512        # feature dim
CT = C // 128  # c-tiles per batch
FC = F // 128  # f-chunks


def build_nc(nb=B):
    nc = bacc.Bacc("TRN2", target_bir_lowering=False, debug=False)
    xc = nc.dram_tensor("x_context", [nb, C, F], f16, kind="ExternalInput").ap()
    xq_d = nc.dram_tensor("x_query", [nb, Q, F], f16, kind="ExternalInput").ap()
    wc_d = nc.dram_tensor("w_context", [F], f16, kind="ExternalInput").ap()
    wcq_d = nc.dram_tensor("w_cq", [F], f32, kind="ExternalInput").ap()
    out = nc.dram_tensor("out", [nb, C, 3 * F], f16, kind="ExternalOutput").ap()

    from contextlib import ExitStack

    with tile.TileContext(nc) as tc, ExitStack() as es:
        def pool(name, bufs, space="SBUF"):
            return es.enter_context(tc.tile_pool(name=name, bufs=bufs, space=space))

        const = pool("const", 1)
        ctx_p = pool("ctx_p", 2)
        ctxT_p = pool("ctxT_p", 2)
        xq_p = pool("xq_p", 2)
        xqw_p = pool("xqw_p", 2)
        tmp_p = pool("tmp_p", 2)
        e_p = pool("e_p", CT + 2)
        et_p = pool("et_p", 3)
        asm_p = pool("asm_p", 2)
        vec_p = pool("vec_p", 3)
        sml_p = pool("sml_p", 2)
        ps_sim_p = pool("ps_sim", 2, "PSUM")
        ps_ctxT_p = pool("ps_ctxT", 2, "PSUM")
        ps_c2q_p = pool("ps_c2q", 2, "PSUM")
        ps_sml_p = pool("ps_sml", 2, "PSUM")

        # loads on the scalar HWDGE ring; stores on the sync (SP) ring
        dma_load = nc.scalar.dma_start
        dma_store = nc.sync.dma_start

        ident = const.tile([128, 128], f16)
        make_identity(nc, ident)
        ones_row = const.tile([1, 128], f16)
        nc.vector.memset(ones_row, 1.0)
        ones_col = const.tile([128, 1], f32)
        nc.vector.memset(ones_col, 1.0)
        wc_sb = const.tile([128, FC], f16)
        dma_load(wc_sb, wc_d.rearrange("(a p) -> p a", p=128))
        wcq_sb = const.tile([128, FC], f32)
        dma_load(wcq_sb, wcq_d.rearrange("(a p) -> p a", p=128))
        wc_row = const.tile([1, F], f16)
        dma_load(wc_row, wc_d[None, :])
        # wc broadcast along partitions (for s_qry): ones[1,128]^T @ wc[1,512]
        ps_wcb = ps_c2q_p.tile([128, F], f32, tag="c2q")
        nc.tensor.matmul(ps_wcb, lhsT=ones_row, rhs=wc_row, start=True, stop=True)
        wc_bc = const.tile([128, F], f16)
        nc.vector.tensor_copy(wc_bc, ps_wcb)

        def load_batch(b):
            ctx = ctx_p.tile([128, CT, F], f16, name="ctx")
            dma_load(ctx, xc[b].rearrange("(t p) f -> p t f", p=128))
            xq = xq_p.tile([128, F], f16, name="xq")
            dma_load(xq, xq_d[b])
            return ctx, xq

        nxt = load_batch(0)
        for b in range(nb):
            # ---- loads (prefetched one batch ahead, before this batch's stores) ----
            ctx, xq = nxt
            if b + 1 < nb:
                nxt = load_batch(b + 1)

            # ---- xqT (one psum bank), scaled by w_cq via one broadcast mul ----
            # xqw_aug[:, fc] = [wcq*xqT chunk | wc chunk]   ([128, 129])
            xqw_aug = xqw_p.tile([128, FC, Q + 1], f16)
            ps_xqT = ps_sml_p.tile([128, FC, 128], f16, tag="sml")
            for fc in range(FC):
                nc.tensor.transpose(
                    ps_xqT[:, fc], xq[:, fc * 128 : (fc + 1) * 128], ident
                )
            for fc in range(FC):
                nc.scalar.activation(
                    xqw_aug[:, fc, 0:Q], ps_xqT[:, fc], COPY,
                    scale=wcq_sb[:, fc : fc + 1],
                )
            nc.vector.tensor_copy(xqw_aug[:, :, Q : Q + 1], wc_sb[:, :, None])

            # ---- s_qry row [1, 128] (fused mul-reduce, then PE transpose) ----
            scr = tmp_p.tile([128, F], f16, name="scr", tag="scr")
            sq_col = vec_p.tile([128, 1], f32, tag="sqcol")
            nc.vector.tensor_mul(scr, xq, wc_bc)
            nc.vector.reduce_sum(sq_col, scr, axis=AX)
            sq16 = vec_p.tile([128, 1], f16, tag="sq16")
            nc.vector.tensor_copy(sq16, sq_col)
            ps_sqT = ps_sml_p.tile([1, 128], f16, tag="sml")
            nc.tensor.transpose(ps_sqT, sq16, ident)
            sq_row = sml_p.tile([1, 128], f16, name="sq_row", tag="sq_row")
            nc.scalar.copy(sq_row, ps_sqT)

            # ---- ctxT [f, c]: 8 fp16 PE transposes per full psum bank ----
            ctxT = ctxT_p.tile([128, FC, C], f16)
            for fc in range(FC):
                ps_ct = ps_ctxT_p.tile([128, C], f16, tag="ct")
                for t in range(CT):
                    nc.tensor.transpose(
                        ps_ct[:, t * 128 : (t + 1) * 128],
                        ctx[:, t, fc * 128 : (fc + 1) * 128],
                        ident,
                    )
                cp = nc.vector.tensor_copy if fc % 2 == 0 else nc.scalar.copy
                cp(ctxT[:, fc], ps_ct)

            # ---- pass 1: sim + softmax stats, 2 c-tiles per psum bank ----
            z = sml_p.tile([128, CT], f32, name="z", tag="z")
            rsum_all = vec_p.tile([128, CT], f32, tag="rsum")
            Es = []
            for t in range(CT):
                ps_sim = ps_sim_p.tile([128, Q + 1], f32, tag="sim")
                for fc in range(FC):
                    nc.tensor.matmul(
                        ps_sim,
                        lhsT=ctxT[:, fc, t * 128 : t * 128 + 128],
                        rhs=xqw_aug[:, fc],
                        start=(fc == 0),
                        stop=False,
                    )
                nc.tensor.matmul(
                    ps_sim[:, 0:Q], lhsT=ones_row, rhs=sq_row, start=False, stop=True
                )
                nmax = vec_p.tile([128, 1], f32, tag="nmax")
                nc.vector.reduce_max(nmax, ps_sim[:, 0:Q], axis=AX, negate=True)
                E = e_p.tile([128, Q], f16)
                nc.scalar.activation(
                    E, ps_sim[:, 0:Q], EXP, bias=nmax,
                    accum_out=rsum_all[:, t : t + 1],
                )
                nc.vector.tensor_sub(z[:, t : t + 1], ps_sim[:, Q : Q + 1], nmax)
                Es.append(E)
            rcp_all = vec_p.tile([128, CT], f32, tag="rcp")
            nc.vector.reciprocal(rcp_all, rsum_all)

            # ---- q2c softmax prep (off the PE critical path) ----
            zmax = vec_p.tile([128, 1], f32, tag="zmax")
            nc.vector.reduce_max(zmax, z, axis=AX)
            gmax = vec_p.tile([128, 1], f32, tag="gmax")
            nc.gpsimd.partition_all_reduce(
                gmax, zmax, channels=128, reduce_op=bass_isa.ReduceOp.max
            )
            negb = vec_p.tile([128, 1], f32, tag="negb")
            nc.vector.tensor_scalar_mul(negb, gmax, -1.0)
            expz = sml_p.tile([128, CT], f16, name="expz", tag="expz")
            ers = vec_p.tile([128, 1], f32, tag="ers")
            nc.scalar.activation(expz, z, EXP, bias=negb, accum_out=ers)

            # ---- pass 2 (software-pipelined): E^T one tile ahead of c2q ----
            asm = asm_p.tile([128, CT, 3 * F], f16)

            def stage_et(t):
                ps_et = ps_sml_p.tile([128, Q], f16, tag="sml")
                nc.tensor.transpose(ps_et, Es[t], ident)
                ET = et_p.tile([128, Q], f16)
                nc.scalar.copy(ET, ps_et)
                return ET

            def stage_c2q(t, ET):
                ps_c2q = ps_c2q_p.tile([128, F], f32, tag="c2q")
                nc.tensor.matmul(ps_c2q, lhsT=ET, rhs=xq, start=True, stop=True)
                # normalized c2q, fused into the psum->sbuf move
                if t % 2 == 0:
                    nc.scalar.activation(
                        asm[:, t, 0:F], ps_c2q, COPY, scale=rcp_all[:, t : t + 1]
                    )
                else:
                    nc.vector.tensor_scalar_mul(
                        asm[:, t, 0:F], ps_c2q, rcp_all[:, t : t + 1]
                    )

            prev = None
            for t in range(CT):
                ET = stage_et(t)
                if prev is not None:
                    stage_c2q(prev[0], prev[1])
                prev = (t, ET)
            stage_c2q(prev[0], prev[1])

            # ---- q2c weighted sum (PE rank-1 fp16 matmuls) + broadcast ----
            ps_S = ps_sml_p.tile([1, 1], f32, tag="sml")
            nc.tensor.matmul(ps_S, lhsT=ers, rhs=ones_col, start=True, stop=True)
            rS = sml_p.tile([1, 1], f32, name="rS", tag="rS")
            nc.vector.reciprocal(rS, ps_S)
            ps_q2c = ps_sml_p.tile([1, F], f32, tag="sml")
            for t in range(CT):
                nc.tensor.matmul(
                    ps_q2c,
                    lhsT=expz[:, t : t + 1],
                    rhs=ctx[:, t],
                    start=(t == 0),
                    stop=(t == CT - 1),
                )
            xq2c = sml_p.tile([1, F], f16, name="xq2c", tag="xq2c")
            nc.scalar.activation(xq2c, ps_q2c, COPY, scale=rS)
            ps_bc = ps_c2q_p.tile([128, F], f32, tag="c2q")
            nc.tensor.matmul(ps_bc, lhsT=ones_row, rhs=xq2c, start=True, stop=True)
            xq2cb = tmp_p.tile([128, F], f16, name="xq2cb", tag="xq2cb")
            nc.vector.tensor_copy(xq2cb, ps_bc)

            # ---- terms 3+4 as whole-batch DVE ops, then ONE store ----
            nc.vector.tensor_mul(asm[:, :, F : 2 * F], ctx, asm[:, :, 0:F])
            for t in range(CT):
                nc.vector.tensor_mul(asm[:, t, 2 * F : 3 * F], ctx[:, t], xq2cb)
            dma_store(out[b].rearrange("(u p) f -> p u f", p=128), asm)

    nc.compile()
    return nc


_NC = None


def kernel(**inputs):
    global _NC
    if _NC is None:
        _NC = build_nc()
    xc32 = np.ascontiguousarray(np.asarray(inputs["x_context"], dtype=np.float32))
    xq32 = np.asarray(inputs["x_query"], dtype=np.float32)
    xc = xc32.astype(np.float16)
    xq = xq32.astype(np.float16)
    wc = np.asarray(inputs["w_context"], dtype=np.float32).astype(np.float16)
    wcq = np.ascontiguousarray(np.asarray(inputs["w_cq"], dtype=np.float32))
    in_maps = [
        {
            "x_context": np.ascontiguousarray(xc[i * B : (i + 1) * B]),
            "x_query": np.ascontiguousarray(xq[i * B : (i + 1) * B]),
            "w_context": wc,
            "w_cq": wcq,
        }
        for i in range(N_CORES)
    ]
    res = bass_utils.run_bass_kernel_spmd(_NC, in_maps, core_ids=list(range(N_CORES)))
    # term 1 of the reference output is x_context verbatim -> host-assembled;
    # the device ships terms 2-4 as fp16.
    full = np.empty((N, C, 4 * F), dtype=np.float32)
    full[:, :, 0:F] = xc32
    for i in range(N_CORES):
        full[i * B : (i + 1) * B, :, F:] = np.asarray(res.results[i]["out"]).astype(np.float32)
    return full
